# revision 29
# baseline (speedup 1.0000x reference)
"""Causal self-attention on 8 Trainium2 NeuronCores.

Sharding: 8 cores = 4 batches x 2 head-groups (8 heads each).
Each core runs an identical SPMD program:
  - QKV projections for its head group (weights pre-transposed + bf16 on host)
  - causal attention computed in transposed-score layout S^T[s, t] so the
    AV matmul consumes P^T directly (no on-chip transposes at all)
  - softmax denominators come for free from a ones-column appended to V
  - row-sharded Wo projection produces a partial output; the two cores of a
    batch are summed on the host during unsharding.

Schedule: Q is kept in two zero-padded copies (even/odd head rows) so the
QK^T matmuls run with a full K=128 contraction - every matmul in the kernel
then uses the same PE array mode (no mode-switch drains), which lets the
builder interleave QK^T, AV (lagged 2 steps behind the exp) and
projection/output-projection "filler" matmuls into one dense PE stream that
stays busy while ScalarE computes the softmax exps.

Perf notes (vs the first working version):
  - softmax denominator broadcast is a single DRAM bounce (write the
    reciprocal row, read it back with a 0-partition-stride AP) instead of
    two round trips; reciprocals run split across ScalarE/VectorE straight
    from PSUM.
  - input DMAs are batched (3D APs) and issued across sync/scalar/gpsimd
    queues, ordered so chunk-0's projection operands land first.
  - the last chunk's output projection is emitted k-major across all 8 PSUM
    groups so only the 8 final (stop) matmuls wait on the last head-pair's
    normalize; everything else overlaps it.
  - a short burst of dummy matmuls at t=0 warms the PE HAM clock gate so
    the first real matmuls run at 2.4 GHz instead of 1.2.

B=4, T=2048, D=1024, H=16, dh=64.
"""

import numpy as np
import ml_dtypes

B, T, D = 4, 2048, 1024
P = 128
KD = D // P  # 8 contraction tiles for the input dim
HL = 8  # heads per core
HP = HL // 2  # head pairs per core (pair shares a 128-partition tile)
DH = 64
TCH = 512  # t-chunk (psum bank width in fp32)
NC4 = T // TCH  # 4 chunks
NTT = T // P  # 16 t-tiles
AVLAG = 2  # AV trails QK^T by this many s-tiles (hides exp latency)

_CACHE = {}


def _split_waits(nc, mybir, limit=1):
    """walrus in this container accepts at most one sem-wait per instruction;
    hoist extra waits onto preceding NoOps on the same engine."""
    cnt = 0
    for bb in nc.main_func.blocks:
        newlist = []
        for inst in bb.instructions:
            si = inst.sync_info
            if si is not None and len(si.on_wait) > limit:
                waits = list(si.on_wait)
                extra, keep = waits[:-limit], waits[-limit:]
                for w in extra:
                    cnt += 1
                    nop = mybir.InstNoOp(name=f"WSPLIT-{cnt}")
                    nop.engine = inst.engine
                    nop.sync_info = mybir.SyncInfo(on_wait=[w], on_update=[])
                    newlist.append(nop)
                inst.sync_info = mybir.SyncInfo(
                    on_wait=keep, on_update=list(si.on_update)
                )
            newlist.append(inst)
        bb.instructions[:] = newlist
    return cnt


def _build():
    if "nc" in _CACHE:
        return _CACHE["nc"]

    from contextlib import ExitStack

    import concourse.bass as bass
    import concourse.tile as tile
    from concourse import mybir

    f32 = mybir.dt.float32
    bf = mybir.dt.bfloat16
    Exp = mybir.ActivationFunctionType.Exp

    nc = bass.Bass()
    xT = nc.declare_dram_parameter("xT", [D, T], bf, isOutput=False)
    wq = nc.declare_dram_parameter("wq", [D, HL * DH], bf, isOutput=False)
    wk = nc.declare_dram_parameter("wk", [D, HL * DH], bf, isOutput=False)
    wv = nc.declare_dram_parameter("wv", [D, HL * DH], bf, isOutput=False)
    wo = nc.declare_dram_parameter("wo", [HL * DH, D], bf, isOutput=False)
    mk = nc.declare_dram_parameter("mask", [P, P], bf, isOutput=False)
    out = nc.declare_dram_parameter("out", [T, D], f32, isOutput=True)
    # DRAM bounce buffers for the softmax-denominator partition broadcast
    lds = [nc.dram_tensor(f"ld{i}", [2 * TCH], bf) for i in range(HP * NC4)]
    rds = [nc.dram_tensor(f"rd{i}", [2 * TCH], bf) for i in range(HP * NC4)]

    with tile.TileContext(nc) as tc, ExitStack() as ctx:
        psum = ctx.enter_context(tc.tile_pool(name="psum", bufs=1, space="PSUM"))
        per = ctx.enter_context(tc.tile_pool(name="per", bufs=1))

        wq_sb = per.tile([P, KD, HL * DH], bf)
        wk_sb = per.tile([P, KD, HL * DH], bf)
        wv_sb = per.tile([P, KD, HL * DH], bf)
        wo_sb = per.tile([P, HL * DH // P, D], bf)
        mk_sb = per.tile([P, P], bf)
        warm_sb = per.tile([P, TCH], bf)  # uninitialized; PE warmup only
        pw_sb = per.tile([1, 2], f32)  # exp-table prewarm scratch
        ones_sb = per.tile([1, 64], bf)  # K=1 matmul row-broadcast weights
        # Q^T in two zero-padded copies: qt0 has even-head rows (0:64) live,
        # qt1 odd-head rows (64:128); the other half stays zero so QK^T can
        # contract over all 128 partitions in the standard PE mode.
        # All of these are split into per-index tiles (instead of one big
        # tile with an extra axis) so Tile's dependency tracking stays exact.
        qt0_sb = {
            (m, cc): per.tile([P, TCH], bf, name=f"qt0_{m}_{cc}")
            for m in range(HP)
            for cc in range(NC4)
        }
        qt1_sb = {
            (m, cc): per.tile([P, TCH], bf, name=f"qt1_{m}_{cc}")
            for m in range(HP)
            for cc in range(NC4)
        }
        kt_sb = {
            (m, cc): per.tile([P, TCH], bf, name=f"kt_{m}_{cc}")
            for m in range(HP)
            for cc in range(NC4)
        }
        v_sb = [per.tile([P, HL, 66], bf, name=f"v_{tt}") for tt in range(NTT)]
        yt_sb = {
            (m, cc): per.tile([P, TCH], bf, name=f"yt_{m}_{cc}")
            for m in range(HP)
            for cc in range(NC4)
        }

        xT_sb = per.tile([P, KD, T], bf, name="xT_sb")

        # ---- PE warmup: dummy matmuls so the HAM clock gate reaches 8/8
        # before the first real projection matmul.
        nc.vector.memset(warm_sb[:], 0.0)
        for w in range(14):
            pwm = psum.tile([P, TCH], f32, tag="pp", bufs=2, name=f"pwm{w}")
            nc.tensor.matmul(
                out=pwm[:],
                lhsT=warm_sb[:, 0:P],
                rhs=warm_sb[:],
                start=True,
                stop=True,
            )

        # ---- input loads, all on the two HWDGE queues (sync/scalar),
        # issued in need-time order: transfers drain roughly in issue order
        # per queue, so the chunk-0 projection operands get the HBM
        # bandwidth first and the later chunks' bulk follows.
        def dram_ap(t, offset, ap):
            return bass.AP(tensor=t, offset=offset, ap=ap)

        def ld_x(eng, k, c0, c1):
            eng.dma_start(
                out=xT_sb[:, k, c0 * TCH : c1 * TCH],
                in_=dram_ap(
                    xT, k * P * T + c0 * TCH, [[T, P], [1, (c1 - c0) * TCH]]
                ),
            )

        def ld_xq(eng, k4, c0, c1):
            eng.dma_start(
                out=xT_sb[:, k4 : k4 + 4, c0 * TCH : c1 * TCH],
                in_=dram_ap(
                    xT,
                    k4 * P * T + c0 * TCH,
                    [[T, P], [P * T, 4], [1, (c1 - c0) * TCH]],
                ),
            )

        def ld_w(eng, wsrc, wdst, h):
            eng.dma_start(
                out=wdst[:, 4 * h : 4 * h + 4, :],
                in_=dram_ap(
                    wsrc, 4 * h * P * 512, [[512, P], [P * 512, 4], [1, 512]]
                ),
            )

        def ld_wo(eng, h):
            eng.dma_start(
                out=wo_sb[:, 2 * h : 2 * h + 2, :],
                in_=dram_ap(wo, 2 * h * P * D, [[D, P], [P * D, 2], [1, D]]),
            )

        ld_w(nc.sync, wq, wq_sb, 0)
        ld_x(nc.scalar, 0, 0, 1)
        ld_w(nc.sync, wq, wq_sb, 1)
        ld_x(nc.scalar, 1, 0, 1)
        ld_x(nc.sync, 2, 0, 1)
        ld_x(nc.scalar, 4, 0, 1)
        ld_x(nc.sync, 3, 0, 1)
        ld_x(nc.scalar, 5, 0, 1)
        ld_x(nc.sync, 6, 0, 1)
        ld_w(nc.scalar, wk, wk_sb, 0)
        ld_x(nc.sync, 7, 0, 1)
        ld_w(nc.scalar, wk, wk_sb, 1)
        ld_w(nc.sync, wv, wv_sb, 0)
        nc.scalar.dma_start(out=mk_sb[:], in_=mk[:, :])
        ld_w(nc.sync, wv, wv_sb, 1)
        ld_xq(nc.scalar, 4, 1, 2)
        ld_xq(nc.sync, 0, 1, 2)
        ld_xq(nc.scalar, 4, 2, 3)
        ld_xq(nc.sync, 0, 2, 3)
        ld_wo(nc.scalar, 0)
        ld_wo(nc.sync, 1)
        ld_xq(nc.scalar, 4, 3, 4)
        ld_xq(nc.sync, 0, 3, 4)

        # ---- zero/one fills (chunk-0 qt pads now; later chunks are zeroed
        # inside their projection generator so they don't clog the DVE queue
        # during startup)
        for tt in range(NTT):
            nc.vector.memset(v_sb[tt][:, :, 64:65], 1.0)
        nc.vector.memset(ones_sb[:], 1.0)
        for m in range(HP):
            nc.vector.memset(qt0_sb[(m, 0)][64:P, :], 0.0)
            nc.vector.memset(qt1_sb[(m, 0)][0:64, :], 0.0)
        # exp activation-table prewarm (reads the just-memset ones column)
        nc.scalar.activation(
            out=pw_sb[0:1, 0:1], in_=v_sb[0][0:1, 0, 64:65], func=Exp, scale=1.0
        )

        pt_pool = ctx.enter_context(tc.tile_pool(name="ptp", bufs=6))
        ysb_pool = ctx.enter_context(tc.tile_pool(name="ysbp", bufs=2))
        sm_pool = ctx.enter_context(tc.tile_pool(name="smp", bufs=4))
        out_pool = ctx.enter_context(tc.tile_pool(name="outp", bufs=2))

        def gen_proj(cc):
            """QKV projections for chunk cc; yields after each matmul.
            Emits all Q m-groups first, then K, then V, so the startup run
            (cc=0) only needs wq+xT[,:512] to keep the PE busy."""
            tsl = slice(cc * TCH, (cc + 1) * TCH)
            if cc > 0:
                for m in range(HP):
                    nc.vector.memset(qt0_sb[(m, cc)][64:P, :], 0.0)
                    nc.vector.memset(qt1_sb[(m, cc)][0:64, :], 0.0)
            for m in range(HP):
                msl = slice(m * P, (m + 1) * P)
                pq = psum.tile([P, TCH], f32, tag="pp", bufs=2, name=f"pq{cc}_{m}")
                for k in range(KD):
                    nc.tensor.matmul(
                        out=pq[:],
                        lhsT=wq_sb[:, k, msl],
                        rhs=xT_sb[:, k, tsl],
                        start=(k == 0),
                        stop=(k == KD - 1),
                    )
                    yield
                nc.vector.tensor_copy(out=qt0_sb[(m, cc)][0:64, :], in_=pq[0:64, :])
                nc.vector.tensor_copy(out=qt1_sb[(m, cc)][64:P, :], in_=pq[64:P, :])
            for m in range(HP):
                msl = slice(m * P, (m + 1) * P)
                pk = psum.tile([P, TCH], f32, tag="pp", bufs=2, name=f"pk{cc}_{m}")
                for k in range(KD):
                    nc.tensor.matmul(
                        out=pk[:],
                        lhsT=wk_sb[:, k, msl],
                        rhs=xT_sb[:, k, tsl],
                        start=(k == 0),
                        stop=(k == KD - 1),
                    )
                    yield
                nc.vector.tensor_copy(out=kt_sb[(m, cc)][:, :], in_=pk[:])
            for tt in range(4 * cc, 4 * cc + 4):
                pv = psum.tile([P, TCH], f32, tag="pp", bufs=2, name=f"pv{tt}")
                for k in range(KD):
                    nc.tensor.matmul(
                        out=pv[:],
                        lhsT=xT_sb[:, k, tt * P : (tt + 1) * P],
                        rhs=wv_sb[:, k, :],
                        start=(k == 0),
                        stop=(k == KD - 1),
                    )
                    yield
                nc.vector.tensor_copy(
                    out=v_sb[tt][:, :, 0:64],
                    in_=pv.rearrange("p (h d) -> p h d", h=HL),
                )

        def gen_oproj(chunks):
            """Output projection for the given chunks; yields per matmul."""
            for c2 in chunks:
                for tt in range(4 * c2, 4 * c2 + 4):
                    ob = out_pool.tile([P, D], f32, tag="ob", name=f"ob{tt}")
                    for n2 in range(2):
                        po = psum.tile(
                            [P, TCH], f32, tag="pp", bufs=2, name=f"po{tt}_{n2}"
                        )
                        for k in range(HL * DH // P):
                            nc.tensor.matmul(
                                out=po[:],
                                lhsT=yt_sb[(k, c2)][:, (tt - 4 * c2) * P : (tt - 4 * c2 + 1) * P],
                                rhs=wo_sb[:, k, n2 * TCH : (n2 + 1) * TCH],
                                start=(k == 0),
                                stop=(k == HL * DH // P - 1),
                            )
                            yield
                        nc.vector.tensor_copy(
                            out=ob[:, n2 * TCH : (n2 + 1) * TCH], in_=po[:]
                        )
                    nc.sync.dma_start(
                        out=out[tt * P : (tt + 1) * P, :], in_=ob[:]
                    )

        # projections for chunk 0 run unzipped up front, with 4 concurrent
        # PSUM groups k-interleaved so every arriving xT/weight k-slice
        # immediately feeds 4 matmuls (the startup is DMA-bound; 2 groups
        # would stall the PE on the open groups' last k-slices)
        def quad_psum(nm):
            a = psum.tile([P, TCH], f32, tag="pp", bufs=2, name=f"{nm}a")
            b = psum.tile([P, TCH], f32, tag="pp", bufs=2, name=f"{nm}b")
            cde = psum.tile([P, 2 * TCH], f32, tag="ps2", bufs=2, name=f"{nm}c")
            return [a[:], b[:], cde[:, 0:TCH], cde[:, TCH : 2 * TCH]]

        pqs = quad_psum("pq0")
        for k in range(KD):
            for m in range(HP):
                nc.tensor.matmul(
                    out=pqs[m],
                    lhsT=wq_sb[:, k, m * P : (m + 1) * P],
                    rhs=xT_sb[:, k, 0:TCH],
                    start=(k == 0),
                    stop=(k == KD - 1),
                )
        for m in range(HP):
            nc.vector.tensor_copy(out=qt0_sb[(m, 0)][0:64, :], in_=pqs[m][0:64, :])
            nc.vector.tensor_copy(out=qt1_sb[(m, 0)][64:P, :], in_=pqs[m][64:P, :])
        pks = quad_psum("pk0")
        for k in range(KD):
            for m in range(HP):
                nc.tensor.matmul(
                    out=pks[m],
                    lhsT=wk_sb[:, k, m * P : (m + 1) * P],
                    rhs=xT_sb[:, k, 0:TCH],
                    start=(k == 0),
                    stop=(k == KD - 1),
                )
        for m in range(HP):
            nc.vector.tensor_copy(out=kt_sb[(m, 0)][:, :], in_=pks[m][:])
        pvs = quad_psum("pv0")
        for k in range(KD):
            for tt in range(4):
                nc.tensor.matmul(
                    out=pvs[tt],
                    lhsT=xT_sb[:, k, tt * P : (tt + 1) * P],
                    rhs=wv_sb[:, k, :],
                    start=(k == 0),
                    stop=(k == KD - 1),
                )
        for tt in range(4):
            nc.vector.tensor_copy(
                out=v_sb[tt][:, :, 0:64],
                in_=pvs[tt].rearrange("p (h d) -> p h d", h=HL),
            )

        # Global filler queue: a list of (tag, generator) consumed ~2 matmuls
        # per attention step; before attention chunk c its projections must be
        # fully emitted (Tile orders by program order), so drain through the
        # matching tag at each chunk start. O-proj generators are appended as
        # soon as their chunk's attention completes.
        fillq = [(("proj", cc), gen_proj(cc)) for cc in range(1, NC4)]
        # o-proj filler is held back until the last chunk: chunks 0-2 are
        # saturated by projection filler alone, while chunk 3 (16-step
        # blocks, no projections left) otherwise starves and runs at the
        # softmax-exp pace instead of the PE pace
        lateq = []

        def fill(n, allow_late=False):
            done = 0
            while done < n and fillq:
                try:
                    next(fillq[0][1])
                    done += 1
                except StopIteration:
                    fillq.pop(0)
            while allow_late and done < n and lateq:
                try:
                    next(lateq[0])
                    done += 1
                except StopIteration:
                    lateq.pop(0)
            return done

        def drain_through(tag):
            while fillq and any(t == tag for t, _ in fillq):
                try:
                    next(fillq[0][1])
                except StopIteration:
                    fillq.pop(0)

        FILL_PER_STEP = 2

        # ---- attention: per chunk, all head pairs, with filler zipped in ----
        for c in range(NC4):
            n_st = 4 * c + 4
            drain_through(("proj", c))

            # last chunk: run hp=2 last; the final O-proj puts k=2 in the
            # stop position of every psum group so only those 8 matmuls wait
            # on the last normalize chain
            hporder = [3, 0, 1, 2] if c == NC4 - 1 else list(range(HP))
            for hp in hporder:
                pts = {}
                psys = {}

                def emit_av(st, hp=hp, pts=pts, psys=psys, n_st=n_st):
                    pt, base, lo = pts[st]
                    for par in (0, 1):
                        if st == 0:
                            psys[par] = psum.tile(
                                [65, TCH], f32, tag="py", bufs=2, name=f"psy{par}"
                            )
                        nc.tensor.matmul(
                            out=psys[par][:, lo:TCH],
                            lhsT=v_sb[st][:, 2 * hp + par, 0:65],
                            rhs=pt[:, base + par, lo:TCH],
                            start=(st == 0),
                            stop=(st == n_st - 1),
                        )

                for st in range(n_st):
                    kd = st - 4 * c  # >=0 on causal-diagonal s-tiles
                    lo = max(kd, 0) * P
                    pss = psum.tile([P, 2 * TCH], f32, tag="ps2", bufs=2, name="pss")
                    for par, qt in ((0, qt0_sb), (1, qt1_sb)):
                        nc.tensor.matmul(
                            out=pss[:, par * TCH + lo : (par + 1) * TCH],
                            lhsT=kt_sb[(hp, st // 4)][
                                :, (st % 4) * P : (st % 4 + 1) * P
                            ],
                            rhs=qt[(hp, c)][:, lo:TCH],
                            start=True,
                            stop=True,
                        )
                    pt = pt_pool.tile([P, 2, TCH], bf, tag="pt", name="pt")
                    nc.scalar.activation(
                        out=pt[:, :, lo:TCH],
                        in_=pss.rearrange("p (a b) -> p a b", a=2)[:, :, lo:TCH],
                        func=Exp,
                        scale=1.0 / np.sqrt(DH),
                    )
                    if kd >= 0:
                        for par in (0, 1):
                            nc.vector.tensor_mul(
                                pt[:, par, lo : lo + P],
                                pt[:, par, lo : lo + P],
                                mk_sb[:],
                            )
                    pts[st] = (pt, 0, lo)
                    if st >= AVLAG:
                        emit_av(st - AVLAG)
                    fill(FILL_PER_STEP, allow_late=(c == NC4 - 1))
                for st in range(n_st - AVLAG, n_st):
                    emit_av(st)

                # normalize: y^T = psy[0:64] / psy[64] (denominator row).
                if c == NC4 - 1 and hp == hporder[-1]:
                    # Last block: the 4-DMA broadcast chain (~9us latency)
                    # would be fully exposed at the tail, so compute
                    # 1/den = exp(-ln(den)) as a row on ScalarE and fan it
                    # across the 64 y partitions with a K=1 ones-matmul
                    # straight into PSUM (all banks are free by now). The
                    # ln/exp tables cost ~1e-3 relative on 1/16th of the
                    # output - negligible - and the chain drops to ~4us.
                    lnr = sm_pool.tile([1, 2 * TCH], f32, tag="lnr", bufs=1, name="lnr")
                    rrow = sm_pool.tile([1, 2 * TCH], bf, tag="rrow", bufs=1, name="rrow")
                    ysb = ysb_pool.tile([64, 2 * TCH], bf, tag="ysb", name="ysb")
                    rbps = []
                    for par in (0, 1):
                        csl = slice(par * TCH, (par + 1) * TCH)
                        nc.scalar.activation(
                            out=lnr[0:1, csl],
                            in_=psys[par][64:65, :],
                            func=mybir.ActivationFunctionType.Ln,
                            scale=1.0,
                        )
                        nc.scalar.activation(
                            out=rrow[0:1, csl], in_=lnr[0:1, csl], func=Exp, scale=-1.0
                        )
                        rbp = psum.tile([P, TCH], f32, tag="py", bufs=2, name=f"rbp{par}")
                        rbps.append(rbp)
                        nc.tensor.matmul(
                            out=rbp[0:64, :],
                            lhsT=ones_sb[:],
                            rhs=rrow[0:1, csl],
                            start=True,
                            stop=True,
                        )
                        nc.vector.tensor_copy(out=ysb[:, csl], in_=psys[par][0:64, :])
                        nc.vector.tensor_mul(
                            yt_sb[(hp, c)][slice(64 * par, 64 * par + 64), :],
                            ysb[:, csl],
                            rbp[0:64, :],
                        )
                    continue
                # Steady state (chain is hidden under the next block's
                # attention): evict the two denominator rows (par0 on
                # ScalarE, par1 on VectorE - single-partition DVE ops are
                # ~6ns/elem so keep them to copies only), bounce through
                # DRAM reshaped to [128, 8] so the reciprocal runs wide,
                # then bounce again to broadcast across the 64 y
                # partitions. DMAs alternate sync/scalar queues.
                it = hp * NC4 + c
                ld, rd = lds[it], rds[it]
                dn = sm_pool.tile([1, 2 * TCH], bf, tag="dn", bufs=2, name="dn")
                nc.scalar.copy(out=dn[0:1, 0:TCH], in_=psys[0][64:65, :])
                nc.vector.tensor_copy(
                    out=dn[0:1, TCH : 2 * TCH], in_=psys[1][64:65, :]
                )
                nc.sync.dma_start(out=ld[:], in_=dn[0:1, :])
                l128 = sm_pool.tile([P, 8], bf, tag="l128", bufs=2, name="l128")
                nc.scalar.dma_start(
                    out=l128[:], in_=bass.AP(tensor=ld, offset=0, ap=[[8, P], [1, 8]])
                )
                r128 = sm_pool.tile([P, 8], bf, tag="r128", bufs=2, name="r128")
                with nc.allow_low_precision(reason="softmax denom recip in bf16"):
                    nc.vector.reciprocal(out=r128[:], in_=l128[:])
                nc.sync.dma_start(
                    out=bass.AP(tensor=rd, offset=0, ap=[[8, P], [1, 8]]), in_=r128[:]
                )
                ysb = ysb_pool.tile([64, 2 * TCH], bf, tag="ysb", name="ysb")
                nc.scalar.copy(out=ysb[:, 0:TCH], in_=psys[0][0:64, :])
                nc.vector.tensor_copy(out=ysb[:, TCH : 2 * TCH], in_=psys[1][0:64, :])
                rb = sm_pool.tile([64, 2 * TCH], bf, tag="rb", bufs=3, name="rb")
                nc.scalar.dma_start(
                    out=rb[:],
                    in_=bass.AP(tensor=rd, offset=0, ap=[[0, 64], [1, 2 * TCH]]),
                )
                for par in (0, 1):
                    rows = slice(64 * par, 64 * par + 64)
                    nc.vector.tensor_mul(
                        yt_sb[(hp, c)][rows, :],
                        ysb[:, par * TCH : (par + 1) * TCH],
                        rb[:, par * TCH : (par + 1) * TCH],
                    )
            # this chunk's output projection becomes filler for chunk 3
            if c < NC4 - 1:
                lateq.append(gen_oproj([c]))

        # drain remaining filler
        while fill(64, allow_late=True):
            pass

        # ---- final chunk's output projection, k-major across all 8 PSUM
        # groups: k=3,0,1 passes for every group run while the last head
        # pair (hp=2) is still normalizing; only the 8 k=2 stop-matmuls wait.
        gl6 = [(12, 0), (12, 1), (13, 0), (13, 1), (14, 0), (14, 1)]
        gl2 = [(15, 0), (15, 1)]
        s0 = psum.tile([P, TCH], f32, tag="pp", bufs=2, name="fo0")
        s1 = psum.tile([P, TCH], f32, tag="pp", bufs=2, name="fo1")
        sA = psum.tile([P, 2 * TCH], f32, tag="ps2", bufs=2, name="foA")
        sB = psum.tile([P, 2 * TCH], f32, tag="ps2", bufs=2, name="foB")
        slots = [
            s0[:],
            s1[:],
            sA[:, 0:TCH],
            sA[:, TCH : 2 * TCH],
            sB[:, 0:TCH],
            sB[:, TCH : 2 * TCH],
        ]

        def fo_mm(slot, tt, n2, k, start, stop):
            nc.tensor.matmul(
                out=slot,
                lhsT=yt_sb[(k, 3)][:, (tt - 12) * P : (tt - 11) * P],
                rhs=wo_sb[:, k, n2 * TCH : (n2 + 1) * TCH],
                start=start,
                stop=stop,
            )

        for k in (3, 0, 1):
            for g, (tt, n2) in enumerate(gl6):
                fo_mm(slots[g], tt, n2, k, start=(k == 3), stop=False)
        s6 = psum.tile([P, TCH], f32, tag="py", bufs=2, name="fo6")
        s7 = psum.tile([P, TCH], f32, tag="py", bufs=2, name="fo7")
        slots += [s6[:], s7[:]]
        for g, (tt, n2) in enumerate(gl2):
            for k in (3, 0, 1):
                fo_mm(slots[6 + g], tt, n2, k, start=(k == 3), stop=False)
        allg = gl6 + gl2
        for g in (6, 7, 0, 1, 2, 3, 4, 5):
            tt, n2 = allg[g]
            fo_mm(slots[g], tt, n2, 2, start=False, stop=True)
            ob = out_pool.tile([P, TCH], f32, tag="ob2", bufs=4, name=f"fob{g}")
            if g % 2 == 0:
                nc.scalar.copy(out=ob[:], in_=slots[g])
            else:
                nc.vector.tensor_copy(out=ob[:], in_=slots[g])
            (nc.sync if g % 2 == 0 else nc.scalar).dma_start(
                out=out[tt * P : (tt + 1) * P, n2 * TCH : (n2 + 1) * TCH],
                in_=ob[:],
            )

    _split_waits(nc, mybir, 1)
    _CACHE["nc"] = nc
    return nc


def kernel(x, Wq, Wk, Wv, Wo):
    from concourse.bass_utils import run_bass_kernel_spmd

    nc = _build()
    bf16 = ml_dtypes.bfloat16

    band = np.tril(np.ones((P, P), np.float32)).T.astype(bf16)  # band[s,j]=s<=j
    xTs = [np.ascontiguousarray(x[b].T).astype(bf16) for b in range(B)]
    in_maps = []
    for c in range(8):
        b, hg = divmod(c, 2)
        sl = slice(512 * hg, 512 * hg + 512)
        in_maps.append(
            {
                "xT": xTs[b],
                "wq": np.ascontiguousarray(Wq[sl, :].T).astype(bf16),
                "wk": np.ascontiguousarray(Wk[sl, :].T).astype(bf16),
                "wv": np.ascontiguousarray(Wv[sl, :].T).astype(bf16),
                "wo": np.ascontiguousarray(Wo[:, sl].T).astype(bf16),
                "mask": band,
            }
        )
    res = None
    for attempt in range(4):
        try:
            res = run_bass_kernel_spmd(nc, in_maps, list(range(8)))
            break
        except Exception:
            # transient NRT_EXEC_UNIT_UNRECOVERABLE has been observed on the
            # first execution of a freshly loaded NEFF; retry a few times
            if attempt == 3:
                raise
            import time

            time.sleep(3)
    _CACHE["exec_time_ns"] = res.exec_time_ns
    outp = np.empty((B, T, D), np.float32)
    for b in range(B):
        outp[b] = res.results[2 * b]["out"] + res.results[2 * b + 1]["out"]
    return outp


# revision 30
# speedup vs baseline: 1.1612x; 1.1612x over previous
"""Causal self-attention on 8 Trainium2 NeuronCores.

Sharding: 8 cores = 4 batches x 2 head-groups (8 heads each).
Each core runs an identical SPMD program:
  - QKV projections for its head group (weights pre-transposed + bf16 on host)
  - causal attention computed in transposed-score layout S^T[s, t] so the
    AV matmul consumes P^T directly (no on-chip transposes at all)
  - softmax denominators come for free from a ones-column appended to V
  - row-sharded Wo projection produces a partial output; the two cores of a
    batch are summed on the host during unsharding.

Schedule: Q is kept in two zero-padded copies (even/odd head rows) so the
QK^T matmuls run with a full K=128 contraction - every matmul in the kernel
then uses the same PE array mode (no mode-switch drains), which lets the
builder interleave QK^T, AV (lagged 2 steps behind the exp) and
projection/output-projection "filler" matmuls into one dense PE stream that
stays busy while ScalarE computes the softmax exps.

Perf notes (vs the first working version):
  - softmax denominator broadcast is a single DRAM bounce (write the
    reciprocal row, read it back with a 0-partition-stride AP) instead of
    two round trips; reciprocals run split across ScalarE/VectorE straight
    from PSUM.
  - input DMAs are batched (3D APs) and issued across sync/scalar/gpsimd
    queues, ordered so chunk-0's projection operands land first.
  - the last chunk's output projection is emitted k-major across all 8 PSUM
    groups so only the 8 final (stop) matmuls wait on the last head-pair's
    normalize; everything else overlaps it.
  - a short burst of dummy matmuls at t=0 warms the PE HAM clock gate so
    the first real matmuls run at 2.4 GHz instead of 1.2.

B=4, T=2048, D=1024, H=16, dh=64.
"""

import numpy as np
import ml_dtypes

B, T, D = 4, 2048, 1024
P = 128
KD = D // P  # 8 contraction tiles for the input dim
HL = 8  # heads per core
HP = HL // 2  # head pairs per core (pair shares a 128-partition tile)
DH = 64
TCH = 512  # t-chunk (psum bank width in fp32)
NC4 = T // TCH  # 4 chunks
NTT = T // P  # 16 t-tiles
AVLAG = 2  # AV trails QK^T by this many s-tiles (hides exp latency)

_CACHE = {}


def _split_waits(nc, mybir, limit=1):
    """walrus in this container accepts at most one sem-wait per instruction;
    hoist extra waits onto preceding NoOps on the same engine."""
    cnt = 0
    for bb in nc.main_func.blocks:
        newlist = []
        for inst in bb.instructions:
            si = inst.sync_info
            if si is not None and len(si.on_wait) > limit:
                waits = list(si.on_wait)
                extra, keep = waits[:-limit], waits[-limit:]
                for w in extra:
                    cnt += 1
                    nop = mybir.InstNoOp(name=f"WSPLIT-{cnt}")
                    nop.engine = inst.engine
                    nop.sync_info = mybir.SyncInfo(on_wait=[w], on_update=[])
                    newlist.append(nop)
                inst.sync_info = mybir.SyncInfo(
                    on_wait=keep, on_update=list(si.on_update)
                )
            newlist.append(inst)
        bb.instructions[:] = newlist
    return cnt


def _build():
    if "nc" in _CACHE:
        return _CACHE["nc"]

    from contextlib import ExitStack

    import concourse.bass as bass
    import concourse.tile as tile
    from concourse import mybir

    f32 = mybir.dt.float32
    bf = mybir.dt.bfloat16
    Exp = mybir.ActivationFunctionType.Exp

    nc = bass.Bass()
    xT = nc.declare_dram_parameter("xT", [D, T], bf, isOutput=False)
    wq = nc.declare_dram_parameter("wq", [D, HL * DH], bf, isOutput=False)
    wk = nc.declare_dram_parameter("wk", [D, HL * DH], bf, isOutput=False)
    wv = nc.declare_dram_parameter("wv", [D, HL * DH], bf, isOutput=False)
    wo = nc.declare_dram_parameter("wo", [HL * DH, D], bf, isOutput=False)
    mk = nc.declare_dram_parameter("mask", [P, P], bf, isOutput=False)
    out = nc.declare_dram_parameter("out", [T, D], f32, isOutput=True)
    # DRAM bounce buffers for the softmax-denominator partition broadcast
    lds = [nc.dram_tensor(f"ld{i}", [2 * TCH], bf) for i in range(HP * NC4)]
    rds = [nc.dram_tensor(f"rd{i}", [2 * TCH], bf) for i in range(HP * NC4)]

    with tile.TileContext(nc) as tc, ExitStack() as ctx:
        psum = ctx.enter_context(tc.tile_pool(name="psum", bufs=1, space="PSUM"))
        per = ctx.enter_context(tc.tile_pool(name="per", bufs=1))

        wq_sb = per.tile([P, KD, HL * DH], bf)
        wk_sb = per.tile([P, KD, HL * DH], bf)
        wv_sb = per.tile([P, KD, HL * DH], bf)
        wo_sb = per.tile([P, HL * DH // P, D], bf)
        mk_sb = per.tile([P, P], bf)
        warm_sb = per.tile([P, TCH], bf)  # uninitialized; PE warmup only
        pw_sb = per.tile([1, 2], f32)  # exp-table prewarm scratch
        ones_sb = per.tile([1, 64], bf)  # K=1 matmul row-broadcast weights
        # Q^T in two zero-padded copies: qt0 has even-head rows (0:64) live,
        # qt1 odd-head rows (64:128); the other half stays zero so QK^T can
        # contract over all 128 partitions in the standard PE mode.
        # All of these are split into per-index tiles (instead of one big
        # tile with an extra axis) so Tile's dependency tracking stays exact.
        qt0_sb = {
            (m, cc): per.tile([P, TCH], bf, name=f"qt0_{m}_{cc}")
            for m in range(HP)
            for cc in range(NC4)
        }
        qt1_sb = {
            (m, cc): per.tile([P, TCH], bf, name=f"qt1_{m}_{cc}")
            for m in range(HP)
            for cc in range(NC4)
        }
        kt_sb = {
            (m, cc): per.tile([P, TCH], bf, name=f"kt_{m}_{cc}")
            for m in range(HP)
            for cc in range(NC4)
        }
        v_sb = [per.tile([P, HL, 66], bf, name=f"v_{tt}") for tt in range(NTT)]
        yt_sb = {
            (m, cc): per.tile([P, TCH], bf, name=f"yt_{m}_{cc}")
            for m in range(HP)
            for cc in range(NC4)
        }

        xT_sb = per.tile([P, KD, T], bf, name="xT_sb")

        # ---- PE warmup: dummy matmuls so the HAM clock gate reaches 8/8
        # before the first real projection matmul.
        nc.vector.memset(warm_sb[:], 0.0)
        for w in range(14):
            pwm = psum.tile([P, TCH], f32, tag="pp", bufs=2, name=f"pwm{w}")
            nc.tensor.matmul(
                out=pwm[:],
                lhsT=warm_sb[:, 0:P],
                rhs=warm_sb[:],
                start=True,
                stop=True,
            )

        # ---- input loads, all on the two HWDGE queues (sync/scalar),
        # issued in need-time order: transfers drain roughly in issue order
        # per queue, so the chunk-0 projection operands get the HBM
        # bandwidth first and the later chunks' bulk follows.
        def dram_ap(t, offset, ap):
            return bass.AP(tensor=t, offset=offset, ap=ap)

        def ld_x(eng, k, c0, c1):
            eng.dma_start(
                out=xT_sb[:, k, c0 * TCH : c1 * TCH],
                in_=dram_ap(
                    xT, k * P * T + c0 * TCH, [[T, P], [1, (c1 - c0) * TCH]]
                ),
            )

        def ld_xq(eng, k4, c0, c1):
            eng.dma_start(
                out=xT_sb[:, k4 : k4 + 4, c0 * TCH : c1 * TCH],
                in_=dram_ap(
                    xT,
                    k4 * P * T + c0 * TCH,
                    [[T, P], [P * T, 4], [1, (c1 - c0) * TCH]],
                ),
            )

        def ld_w(eng, wsrc, wdst, h):
            eng.dma_start(
                out=wdst[:, 4 * h : 4 * h + 4, :],
                in_=dram_ap(
                    wsrc, 4 * h * P * 512, [[512, P], [P * 512, 4], [1, 512]]
                ),
            )

        def ld_wo(eng, h):
            eng.dma_start(
                out=wo_sb[:, 2 * h : 2 * h + 2, :],
                in_=dram_ap(wo, 2 * h * P * D, [[D, P], [P * D, 2], [1, D]]),
            )

        ld_w(nc.sync, wq, wq_sb, 0)
        ld_x(nc.scalar, 0, 0, 1)
        ld_w(nc.sync, wq, wq_sb, 1)
        ld_x(nc.scalar, 1, 0, 1)
        ld_x(nc.sync, 2, 0, 1)
        ld_x(nc.scalar, 4, 0, 1)
        ld_x(nc.sync, 3, 0, 1)
        ld_x(nc.scalar, 5, 0, 1)
        ld_x(nc.sync, 6, 0, 1)
        ld_w(nc.scalar, wk, wk_sb, 0)
        ld_x(nc.sync, 7, 0, 1)
        ld_w(nc.scalar, wk, wk_sb, 1)
        ld_w(nc.sync, wv, wv_sb, 0)
        nc.scalar.dma_start(out=mk_sb[:], in_=mk[:, :])
        ld_w(nc.sync, wv, wv_sb, 1)
        ld_xq(nc.scalar, 4, 1, 2)
        ld_xq(nc.sync, 0, 1, 2)
        ld_xq(nc.scalar, 4, 2, 3)
        ld_xq(nc.sync, 0, 2, 3)
        ld_wo(nc.scalar, 0)
        ld_wo(nc.sync, 1)
        ld_xq(nc.scalar, 4, 3, 4)
        ld_xq(nc.sync, 0, 3, 4)

        # ---- zero/one fills (chunk-0 qt pads now; later chunks are zeroed
        # inside their projection generator so they don't clog the DVE queue
        # during startup)
        for tt in range(NTT):
            nc.vector.memset(v_sb[tt][:, :, 64:65], 1.0)
        nc.vector.memset(ones_sb[:], 1.0)
        for m in range(HP):
            nc.vector.memset(qt0_sb[(m, 0)][64:P, :], 0.0)
            nc.vector.memset(qt1_sb[(m, 0)][0:64, :], 0.0)
        # exp activation-table prewarm (reads the just-memset ones column)
        nc.scalar.activation(
            out=pw_sb[0:1, 0:1], in_=v_sb[0][0:1, 0, 64:65], func=Exp, scale=1.0
        )

        pt_pool = ctx.enter_context(tc.tile_pool(name="ptp", bufs=6))
        ysb_pool = ctx.enter_context(tc.tile_pool(name="ysbp", bufs=2))
        sm_pool = ctx.enter_context(tc.tile_pool(name="smp", bufs=4))
        out_pool = ctx.enter_context(tc.tile_pool(name="outp", bufs=2))

        def gen_proj(cc):
            """QKV projections for chunk cc; yields after each matmul.
            Emits all Q m-groups first, then K, then V, so the startup run
            (cc=0) only needs wq+xT[,:512] to keep the PE busy."""
            tsl = slice(cc * TCH, (cc + 1) * TCH)
            if cc > 0:
                for m in range(HP):
                    nc.vector.memset(qt0_sb[(m, cc)][64:P, :], 0.0)
                    nc.vector.memset(qt1_sb[(m, cc)][0:64, :], 0.0)
            for m in range(HP):
                msl = slice(m * P, (m + 1) * P)
                pq = psum.tile([P, TCH], f32, tag="pp", bufs=2, name=f"pq{cc}_{m}")
                for k in range(KD):
                    nc.tensor.matmul(
                        out=pq[:],
                        lhsT=wq_sb[:, k, msl],
                        rhs=xT_sb[:, k, tsl],
                        start=(k == 0),
                        stop=(k == KD - 1),
                    )
                    yield
                nc.vector.tensor_copy(out=qt0_sb[(m, cc)][0:64, :], in_=pq[0:64, :])
                nc.vector.tensor_copy(out=qt1_sb[(m, cc)][64:P, :], in_=pq[64:P, :])
            for m in range(HP):
                msl = slice(m * P, (m + 1) * P)
                pk = psum.tile([P, TCH], f32, tag="pp", bufs=2, name=f"pk{cc}_{m}")
                for k in range(KD):
                    nc.tensor.matmul(
                        out=pk[:],
                        lhsT=wk_sb[:, k, msl],
                        rhs=xT_sb[:, k, tsl],
                        start=(k == 0),
                        stop=(k == KD - 1),
                    )
                    yield
                nc.vector.tensor_copy(out=kt_sb[(m, cc)][:, :], in_=pk[:])
            for tt in range(4 * cc, 4 * cc + 4):
                pv = psum.tile([P, TCH], f32, tag="pp", bufs=2, name=f"pv{tt}")
                for k in range(KD):
                    nc.tensor.matmul(
                        out=pv[:],
                        lhsT=xT_sb[:, k, tt * P : (tt + 1) * P],
                        rhs=wv_sb[:, k, :],
                        start=(k == 0),
                        stop=(k == KD - 1),
                    )
                    yield
                nc.vector.tensor_copy(
                    out=v_sb[tt][:, :, 0:64],
                    in_=pv.rearrange("p (h d) -> p h d", h=HL),
                )

        def gen_oproj(chunks):
            """Output projection for the given chunks; yields per matmul."""
            for c2 in chunks:
                for tt in range(4 * c2, 4 * c2 + 4):
                    ob = out_pool.tile([P, D], f32, tag="ob", name=f"ob{tt}")
                    for n2 in range(2):
                        po = psum.tile(
                            [P, TCH], f32, tag="pp", bufs=2, name=f"po{tt}_{n2}"
                        )
                        for k in range(HL * DH // P):
                            nc.tensor.matmul(
                                out=po[:],
                                lhsT=yt_sb[(k, c2)][:, (tt - 4 * c2) * P : (tt - 4 * c2 + 1) * P],
                                rhs=wo_sb[:, k, n2 * TCH : (n2 + 1) * TCH],
                                start=(k == 0),
                                stop=(k == HL * DH // P - 1),
                            )
                            yield
                        nc.vector.tensor_copy(
                            out=ob[:, n2 * TCH : (n2 + 1) * TCH], in_=po[:]
                        )
                    nc.sync.dma_start(
                        out=out[tt * P : (tt + 1) * P, :], in_=ob[:]
                    )

        # projections for chunk 0 run unzipped up front, with 4 concurrent
        # PSUM groups k-interleaved so every arriving xT/weight k-slice
        # immediately feeds 4 matmuls (the startup is DMA-bound; 2 groups
        # would stall the PE on the open groups' last k-slices)
        def quad_psum(nm):
            a = psum.tile([P, TCH], f32, tag="pp", bufs=2, name=f"{nm}a")
            b = psum.tile([P, TCH], f32, tag="pp", bufs=2, name=f"{nm}b")
            cde = psum.tile([P, 2 * TCH], f32, tag="ps2", bufs=2, name=f"{nm}c")
            return [a[:], b[:], cde[:, 0:TCH], cde[:, TCH : 2 * TCH]]

        pqs = quad_psum("pq0")
        for k in range(KD):
            for m in range(HP):
                nc.tensor.matmul(
                    out=pqs[m],
                    lhsT=wq_sb[:, k, m * P : (m + 1) * P],
                    rhs=xT_sb[:, k, 0:TCH],
                    start=(k == 0),
                    stop=(k == KD - 1),
                )
        for m in range(HP):
            nc.vector.tensor_copy(out=qt0_sb[(m, 0)][0:64, :], in_=pqs[m][0:64, :])
            nc.vector.tensor_copy(out=qt1_sb[(m, 0)][64:P, :], in_=pqs[m][64:P, :])
        pks = quad_psum("pk0")
        for k in range(KD):
            for m in range(HP):
                nc.tensor.matmul(
                    out=pks[m],
                    lhsT=wk_sb[:, k, m * P : (m + 1) * P],
                    rhs=xT_sb[:, k, 0:TCH],
                    start=(k == 0),
                    stop=(k == KD - 1),
                )
        for m in range(HP):
            nc.vector.tensor_copy(out=kt_sb[(m, 0)][:, :], in_=pks[m][:])
        pvs = quad_psum("pv0")
        for k in range(KD):
            for tt in range(4):
                nc.tensor.matmul(
                    out=pvs[tt],
                    lhsT=xT_sb[:, k, tt * P : (tt + 1) * P],
                    rhs=wv_sb[:, k, :],
                    start=(k == 0),
                    stop=(k == KD - 1),
                )
        for tt in range(4):
            nc.vector.tensor_copy(
                out=v_sb[tt][:, :, 0:64],
                in_=pvs[tt].rearrange("p (h d) -> p h d", h=HL),
            )

        # Global filler queue: a list of (tag, generator) consumed ~2 matmuls
        # per attention step; before attention chunk c its projections must be
        # fully emitted (Tile orders by program order), so drain through the
        # matching tag at each chunk start. O-proj generators are appended as
        # soon as their chunk's attention completes.
        fillq = [(("proj", cc), gen_proj(cc)) for cc in range(1, NC4)]
        # o-proj filler is held back until the last chunk: chunks 0-2 are
        # saturated by projection filler alone, while chunk 3 (16-step
        # blocks, no projections left) otherwise starves and runs at the
        # softmax-exp pace instead of the PE pace
        lateq = []

        def fill(n, allow_late=False):
            done = 0
            while done < n and fillq:
                try:
                    next(fillq[0][1])
                    done += 1
                except StopIteration:
                    fillq.pop(0)
            while allow_late and done < n and lateq:
                try:
                    next(lateq[0])
                    done += 1
                except StopIteration:
                    lateq.pop(0)
            return done

        def drain_through(tag):
            while fillq and any(t == tag for t, _ in fillq):
                try:
                    next(fillq[0][1])
                except StopIteration:
                    fillq.pop(0)

        FILL_PER_STEP = 2

        # ---- attention: per chunk, all head pairs, with filler zipped in ----
        for c in range(NC4):
            n_st = 4 * c + 4
            drain_through(("proj", c))

            # last chunk: run hp=2 last; the final O-proj puts k=2 in the
            # stop position of every psum group so only those 8 matmuls wait
            # on the last normalize chain
            hporder = [3, 0, 1, 2] if c == NC4 - 1 else list(range(HP))
            for hp in hporder:
                pts = {}
                psys = {}

                def emit_av(st, hp=hp, pts=pts, psys=psys, n_st=n_st):
                    pt, base, lo = pts[st]
                    for par in (0, 1):
                        if st == 0:
                            psys[par] = psum.tile(
                                [65, TCH], f32, tag="py", bufs=2, name=f"psy{par}"
                            )
                        nc.tensor.matmul(
                            out=psys[par][:, lo:TCH],
                            lhsT=v_sb[st][:, 2 * hp + par, 0:65],
                            rhs=pt[:, base + par, lo:TCH],
                            start=(st == 0),
                            stop=(st == n_st - 1),
                        )

                for st in range(n_st):
                    kd = st - 4 * c  # >=0 on causal-diagonal s-tiles
                    lo = max(kd, 0) * P
                    pss = psum.tile([P, 2 * TCH], f32, tag="ps2", bufs=2, name="pss")
                    for par, qt in ((0, qt0_sb), (1, qt1_sb)):
                        nc.tensor.matmul(
                            out=pss[:, par * TCH + lo : (par + 1) * TCH],
                            lhsT=kt_sb[(hp, st // 4)][
                                :, (st % 4) * P : (st % 4 + 1) * P
                            ],
                            rhs=qt[(hp, c)][:, lo:TCH],
                            start=True,
                            stop=True,
                        )
                    pt = pt_pool.tile([P, 2, TCH], bf, tag="pt", name="pt")
                    nc.scalar.activation(
                        out=pt[:, :, lo:TCH],
                        in_=pss.rearrange("p (a b) -> p a b", a=2)[:, :, lo:TCH],
                        func=Exp,
                        scale=1.0 / np.sqrt(DH),
                    )
                    if kd >= 0:
                        for par in (0, 1):
                            nc.vector.tensor_mul(
                                pt[:, par, lo : lo + P],
                                pt[:, par, lo : lo + P],
                                mk_sb[:],
                            )
                    pts[st] = (pt, 0, lo)
                    if st >= AVLAG:
                        emit_av(st - AVLAG)
                    fill(FILL_PER_STEP, allow_late=(c == NC4 - 1))
                for st in range(n_st - AVLAG, n_st):
                    emit_av(st)

                # normalize: y^T = psy[0:64] / psy[64] (denominator row).
                if c == NC4 - 1 and hp == hporder[-1]:
                    # Last block: the 4-DMA broadcast chain (~9us latency)
                    # would be fully exposed at the tail, so compute
                    # 1/den = exp(-ln(den)) as a row on ScalarE and fan it
                    # across the 64 y partitions with a K=1 ones-matmul
                    # straight into PSUM (all banks are free by now). The
                    # ln/exp tables cost ~1e-3 relative on 1/16th of the
                    # output - negligible - and the chain drops to ~4us.
                    lnr = sm_pool.tile([1, 2 * TCH], f32, tag="lnr", bufs=1, name="lnr")
                    rrow = sm_pool.tile([1, 2 * TCH], bf, tag="rrow", bufs=1, name="rrow")
                    ysb = ysb_pool.tile([64, 2 * TCH], bf, tag="ysb", name="ysb")
                    rbps = []
                    for par in (0, 1):
                        csl = slice(par * TCH, (par + 1) * TCH)
                        nc.scalar.activation(
                            out=lnr[0:1, csl],
                            in_=psys[par][64:65, :],
                            func=mybir.ActivationFunctionType.Ln,
                            scale=1.0,
                        )
                        nc.scalar.activation(
                            out=rrow[0:1, csl], in_=lnr[0:1, csl], func=Exp, scale=-1.0
                        )
                        rbp = psum.tile([P, TCH], f32, tag="py", bufs=2, name=f"rbp{par}")
                        rbps.append(rbp)
                        nc.tensor.matmul(
                            out=rbp[0:64, :],
                            lhsT=ones_sb[:],
                            rhs=rrow[0:1, csl],
                            start=True,
                            stop=True,
                        )
                        nc.vector.tensor_copy(out=ysb[:, csl], in_=psys[par][0:64, :])
                        nc.vector.tensor_mul(
                            yt_sb[(hp, c)][slice(64 * par, 64 * par + 64), :],
                            ysb[:, csl],
                            rbp[0:64, :],
                        )
                    continue
                # Steady state (chain is hidden under the next block's
                # attention): evict the two denominator rows (par0 on
                # ScalarE, par1 on VectorE - single-partition DVE ops are
                # ~6ns/elem so keep them to copies only), bounce through
                # DRAM reshaped to [128, 8] so the reciprocal runs wide,
                # then bounce again to broadcast across the 64 y
                # partitions. DMAs alternate sync/scalar queues.
                it = hp * NC4 + c
                ld, rd = lds[it], rds[it]
                dn = sm_pool.tile([1, 2 * TCH], bf, tag="dn", bufs=2, name="dn")
                nc.scalar.copy(out=dn[0:1, 0:TCH], in_=psys[0][64:65, :])
                nc.vector.tensor_copy(
                    out=dn[0:1, TCH : 2 * TCH], in_=psys[1][64:65, :]
                )
                nc.sync.dma_start(out=ld[:], in_=dn[0:1, :])
                l128 = sm_pool.tile([P, 8], bf, tag="l128", bufs=2, name="l128")
                nc.scalar.dma_start(
                    out=l128[:], in_=bass.AP(tensor=ld, offset=0, ap=[[8, P], [1, 8]])
                )
                r128 = sm_pool.tile([P, 8], bf, tag="r128", bufs=2, name="r128")
                with nc.allow_low_precision(reason="softmax denom recip in bf16"):
                    nc.vector.reciprocal(out=r128[:], in_=l128[:])
                nc.sync.dma_start(
                    out=bass.AP(tensor=rd, offset=0, ap=[[8, P], [1, 8]]), in_=r128[:]
                )
                ysb = ysb_pool.tile([64, 2 * TCH], bf, tag="ysb", name="ysb")
                nc.scalar.copy(out=ysb[:, 0:TCH], in_=psys[0][0:64, :])
                nc.vector.tensor_copy(out=ysb[:, TCH : 2 * TCH], in_=psys[1][0:64, :])
                rb = sm_pool.tile([64, 2 * TCH], bf, tag="rb", bufs=3, name="rb")
                nc.scalar.dma_start(
                    out=rb[:],
                    in_=bass.AP(tensor=rd, offset=0, ap=[[0, 64], [1, 2 * TCH]]),
                )
                for par in (0, 1):
                    rows = slice(64 * par, 64 * par + 64)
                    nc.vector.tensor_mul(
                        yt_sb[(hp, c)][rows, :],
                        ysb[:, par * TCH : (par + 1) * TCH],
                        rb[:, par * TCH : (par + 1) * TCH],
                    )
            # this chunk's output projection becomes available filler; only
            # the last one is held back for chunk 3's otherwise-starved
            # 16-step blocks
            if c < NC4 - 1:
                if c == NC4 - 2:
                    lateq.append(gen_oproj([c]))
                else:
                    fillq.append((("oproj", c), gen_oproj([c])))

        # drain remaining filler
        while fill(64, allow_late=True):
            pass

        # ---- final chunk's output projection, k-major across all 8 PSUM
        # groups: k=3,0,1 passes for every group run while the last head
        # pair (hp=2) is still normalizing; only the 8 k=2 stop-matmuls wait.
        gl6 = [(12, 0), (12, 1), (13, 0), (13, 1), (14, 0), (14, 1)]
        gl2 = [(15, 0), (15, 1)]
        s0 = psum.tile([P, TCH], f32, tag="pp", bufs=2, name="fo0")
        s1 = psum.tile([P, TCH], f32, tag="pp", bufs=2, name="fo1")
        sA = psum.tile([P, 2 * TCH], f32, tag="ps2", bufs=2, name="foA")
        sB = psum.tile([P, 2 * TCH], f32, tag="ps2", bufs=2, name="foB")
        slots = [
            s0[:],
            s1[:],
            sA[:, 0:TCH],
            sA[:, TCH : 2 * TCH],
            sB[:, 0:TCH],
            sB[:, TCH : 2 * TCH],
        ]

        def fo_mm(slot, tt, n2, k, start, stop):
            nc.tensor.matmul(
                out=slot,
                lhsT=yt_sb[(k, 3)][:, (tt - 12) * P : (tt - 11) * P],
                rhs=wo_sb[:, k, n2 * TCH : (n2 + 1) * TCH],
                start=start,
                stop=stop,
            )

        for k in (3, 0, 1):
            for g, (tt, n2) in enumerate(gl6):
                fo_mm(slots[g], tt, n2, k, start=(k == 3), stop=False)
        s6 = psum.tile([P, TCH], f32, tag="py", bufs=2, name="fo6")
        s7 = psum.tile([P, TCH], f32, tag="py", bufs=2, name="fo7")
        slots += [s6[:], s7[:]]
        for g, (tt, n2) in enumerate(gl2):
            for k in (3, 0, 1):
                fo_mm(slots[6 + g], tt, n2, k, start=(k == 3), stop=False)
        allg = gl6 + gl2
        for g in (6, 7, 0, 1, 2, 3, 4, 5):
            tt, n2 = allg[g]
            fo_mm(slots[g], tt, n2, 2, start=False, stop=True)
            ob = out_pool.tile([P, TCH], f32, tag="ob2", bufs=4, name=f"fob{g}")
            if g % 2 == 0:
                nc.scalar.copy(out=ob[:], in_=slots[g])
            else:
                nc.vector.tensor_copy(out=ob[:], in_=slots[g])
            (nc.sync if g % 2 == 0 else nc.scalar).dma_start(
                out=out[tt * P : (tt + 1) * P, n2 * TCH : (n2 + 1) * TCH],
                in_=ob[:],
            )

    _split_waits(nc, mybir, 1)
    _CACHE["nc"] = nc
    return nc


def kernel(x, Wq, Wk, Wv, Wo):
    from concourse.bass_utils import run_bass_kernel_spmd

    nc = _build()
    bf16 = ml_dtypes.bfloat16

    band = np.tril(np.ones((P, P), np.float32)).T.astype(bf16)  # band[s,j]=s<=j
    xTs = [np.ascontiguousarray(x[b].T).astype(bf16) for b in range(B)]
    in_maps = []
    for c in range(8):
        b, hg = divmod(c, 2)
        sl = slice(512 * hg, 512 * hg + 512)
        in_maps.append(
            {
                "xT": xTs[b],
                "wq": np.ascontiguousarray(Wq[sl, :].T).astype(bf16),
                "wk": np.ascontiguousarray(Wk[sl, :].T).astype(bf16),
                "wv": np.ascontiguousarray(Wv[sl, :].T).astype(bf16),
                "wo": np.ascontiguousarray(Wo[:, sl].T).astype(bf16),
                "mask": band,
            }
        )
    res = None
    for attempt in range(4):
        try:
            res = run_bass_kernel_spmd(nc, in_maps, list(range(8)))
            break
        except Exception:
            # transient NRT_EXEC_UNIT_UNRECOVERABLE has been observed on the
            # first execution of a freshly loaded NEFF; retry a few times
            if attempt == 3:
                raise
            import time

            time.sleep(3)
    _CACHE["exec_time_ns"] = res.exec_time_ns
    outp = np.empty((B, T, D), np.float32)
    for b in range(B):
        outp[b] = res.results[2 * b]["out"] + res.results[2 * b + 1]["out"]
    return outp


# revision 32
# speedup vs baseline: 1.1897x; 1.0245x over previous
"""Causal self-attention on 8 Trainium2 NeuronCores.

Sharding: 8 cores = 4 batches x 2 head-groups (8 heads each).
Each core runs an identical SPMD program:
  - QKV projections for its head group (weights pre-transposed + bf16 on host)
  - causal attention computed in transposed-score layout S^T[s, t] so the
    AV matmul consumes P^T directly (no on-chip transposes at all)
  - softmax denominators come for free from a ones-column appended to V
  - row-sharded Wo projection produces a partial output; the two cores of a
    batch are summed on the host during unsharding.

Schedule: Q is kept in two zero-padded copies (even/odd head rows) so the
QK^T matmuls run with a full K=128 contraction - every matmul in the kernel
then uses the same PE array mode (no mode-switch drains), which lets the
builder interleave QK^T, AV (lagged 2 steps behind the exp) and
projection/output-projection "filler" matmuls into one dense PE stream that
stays busy while ScalarE computes the softmax exps.

Perf notes (vs the first working version):
  - softmax denominator broadcast is a single DRAM bounce (write the
    reciprocal row, read it back with a 0-partition-stride AP) instead of
    two round trips; reciprocals run split across ScalarE/VectorE straight
    from PSUM.
  - input DMAs are batched (3D APs) and issued across sync/scalar/gpsimd
    queues, ordered so chunk-0's projection operands land first.
  - the last chunk's output projection is emitted k-major across all 8 PSUM
    groups so only the 8 final (stop) matmuls wait on the last head-pair's
    normalize; everything else overlaps it.
  - a short burst of dummy matmuls at t=0 warms the PE HAM clock gate so
    the first real matmuls run at 2.4 GHz instead of 1.2.

B=4, T=2048, D=1024, H=16, dh=64.
"""

import numpy as np
import ml_dtypes

B, T, D = 4, 2048, 1024
P = 128
KD = D // P  # 8 contraction tiles for the input dim
HL = 8  # heads per core
HP = HL // 2  # head pairs per core (pair shares a 128-partition tile)
DH = 64
TCH = 512  # t-chunk (psum bank width in fp32)
NC4 = T // TCH  # 4 chunks
NTT = T // P  # 16 t-tiles
AVLAG = 2  # AV trails QK^T by this many s-tiles (hides exp latency)

_CACHE = {}


def _split_waits(nc, mybir, limit=1):
    """walrus in this container accepts at most one sem-wait per instruction;
    hoist extra waits onto preceding NoOps on the same engine."""
    cnt = 0
    for bb in nc.main_func.blocks:
        newlist = []
        for inst in bb.instructions:
            si = inst.sync_info
            if si is not None and len(si.on_wait) > limit:
                waits = list(si.on_wait)
                extra, keep = waits[:-limit], waits[-limit:]
                for w in extra:
                    cnt += 1
                    nop = mybir.InstNoOp(name=f"WSPLIT-{cnt}")
                    nop.engine = inst.engine
                    nop.sync_info = mybir.SyncInfo(on_wait=[w], on_update=[])
                    newlist.append(nop)
                inst.sync_info = mybir.SyncInfo(
                    on_wait=keep, on_update=list(si.on_update)
                )
            newlist.append(inst)
        bb.instructions[:] = newlist
    return cnt


def _build():
    if "nc" in _CACHE:
        return _CACHE["nc"]

    from contextlib import ExitStack

    import concourse.bass as bass
    import concourse.tile as tile
    from concourse import mybir

    f32 = mybir.dt.float32
    bf = mybir.dt.bfloat16
    Exp = mybir.ActivationFunctionType.Exp

    nc = bass.Bass()
    xT = nc.declare_dram_parameter("xT", [D, T], bf, isOutput=False)
    wq = nc.declare_dram_parameter("wq", [D, HL * DH], bf, isOutput=False)
    wk = nc.declare_dram_parameter("wk", [D, HL * DH], bf, isOutput=False)
    wv = nc.declare_dram_parameter("wv", [D, HL * DH], bf, isOutput=False)
    wo = nc.declare_dram_parameter("wo", [HL * DH, D], bf, isOutput=False)
    mk = nc.declare_dram_parameter("mask", [P, P], bf, isOutput=False)
    out = nc.declare_dram_parameter("out", [T, D], f32, isOutput=True)
    # DRAM bounce buffers for the softmax-denominator partition broadcast
    lds = [nc.dram_tensor(f"ld{i}", [2 * TCH], bf) for i in range(HP * NC4)]
    rds = [nc.dram_tensor(f"rd{i}", [2 * TCH], bf) for i in range(HP * NC4)]

    with tile.TileContext(nc) as tc, ExitStack() as ctx:
        psum = ctx.enter_context(tc.tile_pool(name="psum", bufs=1, space="PSUM"))
        per = ctx.enter_context(tc.tile_pool(name="per", bufs=1))

        wq_sb = per.tile([P, KD, HL * DH], bf)
        wk_sb = per.tile([P, KD, HL * DH], bf)
        wv_sb = per.tile([P, KD, HL * DH], bf)
        wo_sb = per.tile([P, HL * DH // P, D], bf)
        mk_sb = per.tile([P, P], bf)
        warm_sb = per.tile([P, TCH], bf)  # uninitialized; PE warmup only
        pw_sb = per.tile([1, 2], f32)  # exp-table prewarm scratch
        ones_sb = per.tile([1, 64], bf)  # K=1 matmul row-broadcast weights
        # Q^T in two zero-padded copies: qt0 has even-head rows (0:64) live,
        # qt1 odd-head rows (64:128); the other half stays zero so QK^T can
        # contract over all 128 partitions in the standard PE mode.
        # All of these are split into per-index tiles (instead of one big
        # tile with an extra axis) so Tile's dependency tracking stays exact.
        qt0_sb = {
            (m, cc): per.tile([P, TCH], bf, name=f"qt0_{m}_{cc}")
            for m in range(HP)
            for cc in range(NC4)
        }
        qt1_sb = {
            (m, cc): per.tile([P, TCH], bf, name=f"qt1_{m}_{cc}")
            for m in range(HP)
            for cc in range(NC4)
        }
        kt_sb = {
            (m, cc): per.tile([P, TCH], bf, name=f"kt_{m}_{cc}")
            for m in range(HP)
            for cc in range(NC4)
        }
        v_sb = [per.tile([P, HL, 66], bf, name=f"v_{tt}") for tt in range(NTT)]
        yt_sb = {
            (m, cc): per.tile([P, TCH], bf, name=f"yt_{m}_{cc}")
            for m in range(HP)
            for cc in range(NC4)
        }

        xT_sb = per.tile([P, KD, T], bf, name="xT_sb")

        # ---- PE warmup: dummy matmuls so the HAM clock gate reaches 8/8
        # before the first real projection matmul.
        nc.vector.memset(warm_sb[:], 0.0)
        for w in range(14):
            pwm = psum.tile([P, TCH], f32, tag="pp", bufs=2, name=f"pwm{w}")
            nc.tensor.matmul(
                out=pwm[:],
                lhsT=warm_sb[:, 0:P],
                rhs=warm_sb[:],
                start=True,
                stop=True,
            )

        # ---- input loads, all on the two HWDGE queues (sync/scalar),
        # issued in need-time order: transfers drain roughly in issue order
        # per queue, so the chunk-0 projection operands get the HBM
        # bandwidth first and the later chunks' bulk follows.
        def dram_ap(t, offset, ap):
            return bass.AP(tensor=t, offset=offset, ap=ap)

        def ld_x(eng, k, c0, c1):
            eng.dma_start(
                out=xT_sb[:, k, c0 * TCH : c1 * TCH],
                in_=dram_ap(
                    xT, k * P * T + c0 * TCH, [[T, P], [1, (c1 - c0) * TCH]]
                ),
            )

        def ld_xq(eng, k4, c0, c1):
            eng.dma_start(
                out=xT_sb[:, k4 : k4 + 4, c0 * TCH : c1 * TCH],
                in_=dram_ap(
                    xT,
                    k4 * P * T + c0 * TCH,
                    [[T, P], [P * T, 4], [1, (c1 - c0) * TCH]],
                ),
            )

        def ld_w(eng, wsrc, wdst, h):
            eng.dma_start(
                out=wdst[:, 4 * h : 4 * h + 4, :],
                in_=dram_ap(
                    wsrc, 4 * h * P * 512, [[512, P], [P * 512, 4], [1, 512]]
                ),
            )

        def ld_wo(eng, h):
            eng.dma_start(
                out=wo_sb[:, 2 * h : 2 * h + 2, :],
                in_=dram_ap(wo, 2 * h * P * D, [[D, P], [P * D, 2], [1, D]]),
            )

        ld_w(nc.sync, wq, wq_sb, 0)
        ld_x(nc.scalar, 0, 0, 1)
        ld_w(nc.sync, wq, wq_sb, 1)
        ld_x(nc.scalar, 1, 0, 1)
        ld_x(nc.sync, 2, 0, 1)
        ld_x(nc.scalar, 4, 0, 1)
        ld_x(nc.sync, 3, 0, 1)
        ld_x(nc.scalar, 5, 0, 1)
        ld_x(nc.sync, 6, 0, 1)
        ld_w(nc.scalar, wk, wk_sb, 0)
        ld_x(nc.sync, 7, 0, 1)
        ld_w(nc.scalar, wk, wk_sb, 1)
        ld_w(nc.sync, wv, wv_sb, 0)
        nc.scalar.dma_start(out=mk_sb[:], in_=mk[:, :])
        ld_w(nc.sync, wv, wv_sb, 1)
        ld_xq(nc.scalar, 4, 1, 2)
        ld_xq(nc.sync, 0, 1, 2)
        ld_xq(nc.scalar, 4, 2, 3)
        ld_xq(nc.sync, 0, 2, 3)
        ld_wo(nc.scalar, 0)
        ld_wo(nc.sync, 1)
        ld_xq(nc.scalar, 4, 3, 4)
        ld_xq(nc.sync, 0, 3, 4)

        # ---- zero/one fills (chunk-0 qt pads now; later chunks are zeroed
        # inside their projection generator so they don't clog the DVE queue
        # during startup)
        for tt in range(NTT):
            nc.vector.memset(v_sb[tt][:, :, 64:65], 1.0)
        nc.vector.memset(ones_sb[:], 1.0)
        for m in range(HP):
            nc.vector.memset(qt0_sb[(m, 0)][64:P, :], 0.0)
            nc.vector.memset(qt1_sb[(m, 0)][0:64, :], 0.0)
        # exp activation-table prewarm (reads the just-memset ones column)
        nc.scalar.activation(
            out=pw_sb[0:1, 0:1], in_=v_sb[0][0:1, 0, 64:65], func=Exp, scale=1.0
        )

        pt_pool = ctx.enter_context(tc.tile_pool(name="ptp", bufs=6))
        ysb_pool = ctx.enter_context(tc.tile_pool(name="ysbp", bufs=2))
        sm_pool = ctx.enter_context(tc.tile_pool(name="smp", bufs=4))
        out_pool = ctx.enter_context(tc.tile_pool(name="outp", bufs=2))

        def gen_proj(cc):
            """QKV projections for chunk cc; yields after each matmul.
            Emits all Q m-groups first, then K, then V, so the startup run
            (cc=0) only needs wq+xT[,:512] to keep the PE busy."""
            tsl = slice(cc * TCH, (cc + 1) * TCH)
            if cc > 0:
                for m in range(HP):
                    nc.vector.memset(qt0_sb[(m, cc)][64:P, :], 0.0)
                    nc.vector.memset(qt1_sb[(m, cc)][0:64, :], 0.0)
            for m in range(HP):
                msl = slice(m * P, (m + 1) * P)
                pq = psum.tile([P, TCH], f32, tag="pp", bufs=2, name=f"pq{cc}_{m}")
                for k in range(KD):
                    nc.tensor.matmul(
                        out=pq[:],
                        lhsT=wq_sb[:, k, msl],
                        rhs=xT_sb[:, k, tsl],
                        start=(k == 0),
                        stop=(k == KD - 1),
                    )
                    yield
                nc.vector.tensor_copy(out=qt0_sb[(m, cc)][0:64, :], in_=pq[0:64, :])
                nc.vector.tensor_copy(out=qt1_sb[(m, cc)][64:P, :], in_=pq[64:P, :])
            for m in range(HP):
                msl = slice(m * P, (m + 1) * P)
                pk = psum.tile([P, TCH], f32, tag="pp", bufs=2, name=f"pk{cc}_{m}")
                for k in range(KD):
                    nc.tensor.matmul(
                        out=pk[:],
                        lhsT=wk_sb[:, k, msl],
                        rhs=xT_sb[:, k, tsl],
                        start=(k == 0),
                        stop=(k == KD - 1),
                    )
                    yield
                nc.vector.tensor_copy(out=kt_sb[(m, cc)][:, :], in_=pk[:])
            for tt in range(4 * cc, 4 * cc + 4):
                pv = psum.tile([P, TCH], f32, tag="pp", bufs=2, name=f"pv{tt}")
                for k in range(KD):
                    nc.tensor.matmul(
                        out=pv[:],
                        lhsT=xT_sb[:, k, tt * P : (tt + 1) * P],
                        rhs=wv_sb[:, k, :],
                        start=(k == 0),
                        stop=(k == KD - 1),
                    )
                    yield
                nc.vector.tensor_copy(
                    out=v_sb[tt][:, :, 0:64],
                    in_=pv.rearrange("p (h d) -> p h d", h=HL),
                )

        def gen_oproj(chunks):
            """Output projection for the given chunks; yields per matmul."""
            for c2 in chunks:
                for tt in range(4 * c2, 4 * c2 + 4):
                    ob = out_pool.tile([P, D], f32, tag="ob", name=f"ob{tt}")
                    for n2 in range(2):
                        po = psum.tile(
                            [P, TCH], f32, tag="pp", bufs=2, name=f"po{tt}_{n2}"
                        )
                        for k in range(HL * DH // P):
                            nc.tensor.matmul(
                                out=po[:],
                                lhsT=yt_sb[(k, c2)][:, (tt - 4 * c2) * P : (tt - 4 * c2 + 1) * P],
                                rhs=wo_sb[:, k, n2 * TCH : (n2 + 1) * TCH],
                                start=(k == 0),
                                stop=(k == HL * DH // P - 1),
                            )
                            yield
                        nc.vector.tensor_copy(
                            out=ob[:, n2 * TCH : (n2 + 1) * TCH], in_=po[:]
                        )
                    nc.sync.dma_start(
                        out=out[tt * P : (tt + 1) * P, :], in_=ob[:]
                    )

        # projections for chunk 0 run unzipped up front, with 4 concurrent
        # PSUM groups k-interleaved so every arriving xT/weight k-slice
        # immediately feeds 4 matmuls (the startup is DMA-bound; 2 groups
        # would stall the PE on the open groups' last k-slices)
        def quad_psum(nm):
            a = psum.tile([P, TCH], f32, tag="pp", bufs=2, name=f"{nm}a")
            b = psum.tile([P, TCH], f32, tag="pp", bufs=2, name=f"{nm}b")
            cde = psum.tile([P, 2 * TCH], f32, tag="ps2", bufs=2, name=f"{nm}c")
            return [a[:], b[:], cde[:, 0:TCH], cde[:, TCH : 2 * TCH]]

        pqs = quad_psum("pq0")
        for k in range(KD):
            for m in range(HP):
                nc.tensor.matmul(
                    out=pqs[m],
                    lhsT=wq_sb[:, k, m * P : (m + 1) * P],
                    rhs=xT_sb[:, k, 0:TCH],
                    start=(k == 0),
                    stop=(k == KD - 1),
                )
        for m in range(HP):
            nc.vector.tensor_copy(out=qt0_sb[(m, 0)][0:64, :], in_=pqs[m][0:64, :])
            nc.vector.tensor_copy(out=qt1_sb[(m, 0)][64:P, :], in_=pqs[m][64:P, :])
        pks = quad_psum("pk0")
        for k in range(KD):
            for m in range(HP):
                nc.tensor.matmul(
                    out=pks[m],
                    lhsT=wk_sb[:, k, m * P : (m + 1) * P],
                    rhs=xT_sb[:, k, 0:TCH],
                    start=(k == 0),
                    stop=(k == KD - 1),
                )
        for m in range(HP):
            nc.vector.tensor_copy(out=kt_sb[(m, 0)][:, :], in_=pks[m][:])
        pvs = quad_psum("pv0")
        for k in range(KD):
            for tt in range(4):
                nc.tensor.matmul(
                    out=pvs[tt],
                    lhsT=xT_sb[:, k, tt * P : (tt + 1) * P],
                    rhs=wv_sb[:, k, :],
                    start=(k == 0),
                    stop=(k == KD - 1),
                )
        for tt in range(4):
            nc.vector.tensor_copy(
                out=v_sb[tt][:, :, 0:64],
                in_=pvs[tt].rearrange("p (h d) -> p h d", h=HL),
            )

        # Global filler queue: a list of (tag, generator) consumed ~2 matmuls
        # per attention step; before attention chunk c its projections must be
        # fully emitted (Tile orders by program order), so drain through the
        # matching tag at each chunk start. O-proj generators are appended as
        # soon as their chunk's attention completes.
        fillq = [(("proj", cc), gen_proj(cc)) for cc in range(1, NC4)]
        # o-proj filler is held back until the last chunk: chunks 0-2 are
        # saturated by projection filler alone, while chunk 3 (16-step
        # blocks, no projections left) otherwise starves and runs at the
        # softmax-exp pace instead of the PE pace
        lateq = []

        def fill(n, allow_late=False):
            done = 0
            while done < n and fillq:
                try:
                    next(fillq[0][1])
                    done += 1
                except StopIteration:
                    fillq.pop(0)
            while allow_late and done < n and lateq:
                try:
                    next(lateq[0])
                    done += 1
                except StopIteration:
                    lateq.pop(0)
            return done

        def drain_through(tag):
            while fillq and any(t == tag for t, _ in fillq):
                try:
                    next(fillq[0][1])
                except StopIteration:
                    fillq.pop(0)

        FILL_PER_STEP = 2

        # ---- attention: per chunk, all head pairs, with filler zipped in ----
        for c in range(NC4):
            n_st = 4 * c + 4
            drain_through(("proj", c))

            # last chunk: run hp=2 last; the final O-proj puts k=2 in the
            # stop position of every psum group so only those 8 matmuls wait
            # on the last normalize chain
            hporder = [3, 0, 1, 2] if c == NC4 - 1 else list(range(HP))
            for hp in hporder:
                pts = {}
                psys = {}

                def emit_av(st, hp=hp, pts=pts, psys=psys, n_st=n_st):
                    pt, base, lo = pts[st]
                    for par in (0, 1):
                        if st == 0:
                            psys[par] = psum.tile(
                                [65, TCH], f32, tag="py", bufs=2, name=f"psy{par}"
                            )
                        nc.tensor.matmul(
                            out=psys[par][:, lo:TCH],
                            lhsT=v_sb[st][:, 2 * hp + par, 0:65],
                            rhs=pt[:, base + par, lo:TCH],
                            start=(st == 0),
                            stop=(st == n_st - 1),
                        )

                for st in range(n_st):
                    kd = st - 4 * c  # >=0 on causal-diagonal s-tiles
                    lo = max(kd, 0) * P
                    pss = psum.tile([P, 2 * TCH], f32, tag="ps2", bufs=2, name="pss")
                    for par, qt in ((0, qt0_sb), (1, qt1_sb)):
                        nc.tensor.matmul(
                            out=pss[:, par * TCH + lo : (par + 1) * TCH],
                            lhsT=kt_sb[(hp, st // 4)][
                                :, (st % 4) * P : (st % 4 + 1) * P
                            ],
                            rhs=qt[(hp, c)][:, lo:TCH],
                            start=True,
                            stop=True,
                        )
                    pt = pt_pool.tile([P, 2, TCH], bf, tag="pt", name="pt")
                    nc.scalar.activation(
                        out=pt[:, :, lo:TCH],
                        in_=pss.rearrange("p (a b) -> p a b", a=2)[:, :, lo:TCH],
                        func=Exp,
                        scale=1.0 / np.sqrt(DH),
                    )
                    if kd >= 0:
                        for par in (0, 1):
                            nc.vector.tensor_mul(
                                pt[:, par, lo : lo + P],
                                pt[:, par, lo : lo + P],
                                mk_sb[:],
                            )
                    pts[st] = (pt, 0, lo)
                    if st >= AVLAG:
                        emit_av(st - AVLAG)
                    fill(FILL_PER_STEP, allow_late=(c == NC4 - 1))
                for st in range(n_st - AVLAG, n_st):
                    emit_av(st)

                # normalize: y^T = psy[0:64] / psy[64] (denominator row).
                if c == NC4 - 1 and hp == hporder[-1]:
                    # Last block: the 4-DMA broadcast chain (~9us latency)
                    # would be fully exposed at the tail, so compute
                    # 1/den = exp(-ln(den)) as a row on ScalarE and fan it
                    # across the 64 y partitions with a K=1 ones-matmul
                    # straight into PSUM (all banks are free by now). The
                    # ln/exp tables cost ~1e-3 relative on 1/16th of the
                    # output - negligible - and the chain drops to ~4us.
                    lnr = sm_pool.tile([1, 2 * TCH], f32, tag="lnr", bufs=1, name="lnr")
                    rrow = sm_pool.tile([1, 2 * TCH], bf, tag="rrow", bufs=1, name="rrow")
                    ysb = ysb_pool.tile([64, 2 * TCH], bf, tag="ysb", name="ysb")
                    rbps = []
                    for par in (0, 1):
                        csl = slice(par * TCH, (par + 1) * TCH)
                        nc.scalar.activation(
                            out=lnr[0:1, csl],
                            in_=psys[par][64:65, :],
                            func=mybir.ActivationFunctionType.Ln,
                            scale=1.0,
                        )
                        nc.scalar.activation(
                            out=rrow[0:1, csl], in_=lnr[0:1, csl], func=Exp, scale=-1.0
                        )
                        rbp = psum.tile([P, TCH], f32, tag="py", bufs=2, name=f"rbp{par}")
                        rbps.append(rbp)
                        nc.tensor.matmul(
                            out=rbp[0:64, :],
                            lhsT=ones_sb[:],
                            rhs=rrow[0:1, csl],
                            start=True,
                            stop=True,
                        )
                        nc.vector.tensor_copy(out=ysb[:, csl], in_=psys[par][0:64, :])
                        nc.vector.tensor_mul(
                            yt_sb[(hp, c)][slice(64 * par, 64 * par + 64), :],
                            ysb[:, csl],
                            rbp[0:64, :],
                        )
                    continue
                # Steady state (chain is hidden under the next block's
                # attention): evict the two denominator rows (par0 on
                # ScalarE, par1 on VectorE - single-partition DVE ops are
                # ~6ns/elem so keep them to copies only), bounce through
                # DRAM reshaped to [128, 8] so the reciprocal runs wide,
                # then bounce again to broadcast across the 64 y
                # partitions. DMAs alternate sync/scalar queues.
                it = hp * NC4 + c
                ld, rd = lds[it], rds[it]
                dn = sm_pool.tile([1, 2 * TCH], bf, tag="dn", bufs=2, name="dn")
                nc.scalar.copy(out=dn[0:1, 0:TCH], in_=psys[0][64:65, :])
                nc.vector.tensor_copy(
                    out=dn[0:1, TCH : 2 * TCH], in_=psys[1][64:65, :]
                )
                nc.sync.dma_start(out=ld[:], in_=dn[0:1, :])
                l128 = sm_pool.tile([P, 8], bf, tag="l128", bufs=2, name="l128")
                nc.scalar.dma_start(
                    out=l128[:], in_=bass.AP(tensor=ld, offset=0, ap=[[8, P], [1, 8]])
                )
                r128 = sm_pool.tile([P, 8], bf, tag="r128", bufs=2, name="r128")
                with nc.allow_low_precision(reason="softmax denom recip in bf16"):
                    nc.vector.reciprocal(out=r128[:], in_=l128[:])
                nc.sync.dma_start(
                    out=bass.AP(tensor=rd, offset=0, ap=[[8, P], [1, 8]]), in_=r128[:]
                )
                ysb = ysb_pool.tile([64, 2 * TCH], bf, tag="ysb", name="ysb")
                nc.scalar.copy(out=ysb[:, 0:TCH], in_=psys[0][0:64, :])
                nc.vector.tensor_copy(out=ysb[:, TCH : 2 * TCH], in_=psys[1][0:64, :])
                rb = sm_pool.tile([64, 2 * TCH], bf, tag="rb", bufs=3, name="rb")
                nc.scalar.dma_start(
                    out=rb[:],
                    in_=bass.AP(tensor=rd, offset=0, ap=[[0, 64], [1, 2 * TCH]]),
                )
                for par in (0, 1):
                    rows = slice(64 * par, 64 * par + 64)
                    nc.vector.tensor_mul(
                        yt_sb[(hp, c)][rows, :],
                        ysb[:, par * TCH : (par + 1) * TCH],
                        rb[:, par * TCH : (par + 1) * TCH],
                    )
            # this chunk's output projection becomes available filler
            if c < NC4 - 1:
                fillq.append((("oproj", c), gen_oproj([c])))

        # drain remaining filler
        while fill(64, allow_late=True):
            pass

        # ---- final chunk's output projection, k-major across all 8 PSUM
        # groups: k=3,0,1 passes for every group run while the last head
        # pair (hp=2) is still normalizing; only the 8 k=2 stop-matmuls wait.
        gl6 = [(12, 0), (12, 1), (13, 0), (13, 1), (14, 0), (14, 1)]
        gl2 = [(15, 0), (15, 1)]
        s0 = psum.tile([P, TCH], f32, tag="pp", bufs=2, name="fo0")
        s1 = psum.tile([P, TCH], f32, tag="pp", bufs=2, name="fo1")
        sA = psum.tile([P, 2 * TCH], f32, tag="ps2", bufs=2, name="foA")
        sB = psum.tile([P, 2 * TCH], f32, tag="ps2", bufs=2, name="foB")
        slots = [
            s0[:],
            s1[:],
            sA[:, 0:TCH],
            sA[:, TCH : 2 * TCH],
            sB[:, 0:TCH],
            sB[:, TCH : 2 * TCH],
        ]

        def fo_mm(slot, tt, n2, k, start, stop):
            nc.tensor.matmul(
                out=slot,
                lhsT=yt_sb[(k, 3)][:, (tt - 12) * P : (tt - 11) * P],
                rhs=wo_sb[:, k, n2 * TCH : (n2 + 1) * TCH],
                start=start,
                stop=stop,
            )

        for k in (3, 0, 1):
            for g, (tt, n2) in enumerate(gl6):
                fo_mm(slots[g], tt, n2, k, start=(k == 3), stop=False)
        s6 = psum.tile([P, TCH], f32, tag="py", bufs=2, name="fo6")
        s7 = psum.tile([P, TCH], f32, tag="py", bufs=2, name="fo7")
        slots += [s6[:], s7[:]]
        for g, (tt, n2) in enumerate(gl2):
            for k in (3, 0, 1):
                fo_mm(slots[6 + g], tt, n2, k, start=(k == 3), stop=False)
        fobs = {}
        for g, (tt, n2) in enumerate(gl6 + gl2):
            fo_mm(slots[g], tt, n2, 2, start=False, stop=True)
            if tt not in fobs:
                fobs[tt] = out_pool.tile(
                    [P, D], f32, tag="ob2", bufs=4, name=f"fob{tt}"
                )
            ob = fobs[tt]
            dst = ob[:, n2 * TCH : (n2 + 1) * TCH]
            if g % 2 == 0:
                nc.scalar.copy(out=dst, in_=slots[g])
            else:
                nc.vector.tensor_copy(out=dst, in_=slots[g])
            if n2 == 1:
                (nc.sync if tt % 2 == 0 else nc.scalar).dma_start(
                    out=out[tt * P : (tt + 1) * P, :], in_=ob[:]
                )

    _split_waits(nc, mybir, 1)
    _CACHE["nc"] = nc
    return nc


def kernel(x, Wq, Wk, Wv, Wo):
    from concourse.bass_utils import run_bass_kernel_spmd

    nc = _build()
    bf16 = ml_dtypes.bfloat16

    band = np.tril(np.ones((P, P), np.float32)).T.astype(bf16)  # band[s,j]=s<=j
    xTs = [np.ascontiguousarray(x[b].T).astype(bf16) for b in range(B)]
    in_maps = []
    for c in range(8):
        b, hg = divmod(c, 2)
        sl = slice(512 * hg, 512 * hg + 512)
        in_maps.append(
            {
                "xT": xTs[b],
                "wq": np.ascontiguousarray(Wq[sl, :].T).astype(bf16),
                "wk": np.ascontiguousarray(Wk[sl, :].T).astype(bf16),
                "wv": np.ascontiguousarray(Wv[sl, :].T).astype(bf16),
                "wo": np.ascontiguousarray(Wo[:, sl].T).astype(bf16),
                "mask": band,
            }
        )
    res = None
    for attempt in range(4):
        try:
            res = run_bass_kernel_spmd(nc, in_maps, list(range(8)))
            break
        except Exception:
            # transient NRT_EXEC_UNIT_UNRECOVERABLE has been observed on the
            # first execution of a freshly loaded NEFF; retry a few times
            if attempt == 3:
                raise
            import time

            time.sleep(3)
    _CACHE["exec_time_ns"] = res.exec_time_ns
    outp = np.empty((B, T, D), np.float32)
    for b in range(B):
        outp[b] = res.results[2 * b]["out"] + res.results[2 * b + 1]["out"]
    return outp
